# revision 7
# baseline (speedup 1.0000x reference)
"""DirSageConv Trainium2 kernel (8 NeuronCores, SPMD).

Strategy: nodes sharded across 8 cores (dst for the "in" direction, src for
the "out" direction); edges partitioned by the aggregation key so the
segment-sum is core-local. The host prepares, per core and direction, an
edge-payload stream (features of the gathered endpoint, bf16, sorted by
aggregation tile and padded per-tile to 128-slot chunks). The device streams
the payload and computes segment-sums as one-hot matmuls accumulated in PSUM
(lhsT = payload chunk [128e x 64f], rhs = one-hot [128e x 128dst] built on
the vector engine via is_equal against an iota row), then multiplies by the
weight matrix. Degree division, bias add, and the outer ELU commute with the
linear ops and are applied on the host. The self branch (two matmuls with an
inner ELU) runs on-device per 512-node chunk; its bias/outer-ELU also finish
on host.
"""
import sys

sys.path.insert(0, "/opt/trn_rl_repo")

import ml_dtypes
import numpy as np

import concourse.bacc as bacc
import concourse.mybir as mybir
from concourse import tile
from concourse.bass_utils import run_bass_kernel_spmd

F32 = mybir.dt.float32
BF16 = mybir.dt.bfloat16

AF = mybir.ActivationFunctionType
ALU = mybir.AluOpType

BF = ml_dtypes.bfloat16


class P:
    def __init__(self, N=100000, F_IN=64, F_OUT=128, F_HID=512, NCORES=8):
        self.N, self.F_IN, self.F_OUT, self.F_HID = N, F_IN, F_OUT, F_HID
        self.NCORES = NCORES
        self.NPC = N // NCORES                    # nodes per core
        self.TPC = -(-self.NPC // 128)            # dst tiles per core (98)
        self.NPAD = self.TPC * 128                # padded nodes per core
        self.GT = 4                               # tiles per psum group
        self.NGRP = -(-self.TPC // self.GT)
        self.SELF_CH = -(-self.NPAD // 512)       # self-branch 512-row chunks
        self.XS_ROWS = 512 * self.SELF_CH


def prep_dir(key, val, x16, p: P):
    """Host prep for one direction: payload stream + one-hot dst columns.

    key: aggregation index per edge (node that receives the sum)
    val: gathered node per edge (features fed into the sum)
    Returns (pay_w [C,128,NCH,64] bf16, dstl_w [C,128,NCH] bf16, CH chunks
    per tile, recip [N] f32).
    """
    E = key.shape[0]
    core = np.minimum(key // p.NPC, p.NCORES - 1)
    kl = key - core * p.NPC
    t = kl >> 7
    dl = (kl & 127).astype(np.float32)

    seg = core * p.TPC + t
    cnt = np.bincount(seg, minlength=p.NCORES * p.TPC)
    CH = int(-(-cnt.max() // 128))                # chunks per tile, uniform
    CAP = CH * 128
    NCH = p.TPC * CH

    order = np.argsort(seg, kind="stable")
    seg_s = seg[order]
    m = np.empty(E, np.bool_)
    m[0] = True
    np.not_equal(seg_s[1:], seg_s[:-1], out=m[1:])
    starts = np.flatnonzero(m)
    sid = np.cumsum(m) - 1
    rank_s = np.arange(E) - starts[sid]
    rank = np.empty(E, np.int64)
    rank[order] = rank_s

    slot = core * (p.TPC * CAP) + t * CAP + rank  # global slot
    pay = np.zeros((p.NCORES * p.TPC * CAP, p.F_IN), BF)
    pay[slot] = x16[val]
    dstl = np.full(p.NCORES * p.TPC * CAP, 255.0, np.float32)
    dstl[slot] = dl

    pay_w = pay.reshape(p.NCORES, NCH, 128, p.F_IN).transpose(0, 2, 1, 3)
    dstl_w = dstl.reshape(p.NCORES, NCH, 128).transpose(0, 2, 1)

    deg = np.bincount(key, minlength=p.N).astype(np.float32)
    recip = 1.0 / np.maximum(deg, 1.0)
    return np.ascontiguousarray(pay_w), np.ascontiguousarray(dstl_w), CH, recip


def _emit_elu(nc, pool, psum_ap, out_tile, n, out_dtype):
    """out = elu(psum) = min(exp(z)-1, relu(z)); psum [128, n]."""
    e = pool.tile([128, n], out_dtype, tag="elu_e")
    nc.scalar.activation(e[:, :n], psum_ap, AF.Exp)
    r = pool.tile([128, n], out_dtype, tag="elu_r")
    nc.vector.tensor_scalar_max(r[:, :n], psum_ap, 0.0)
    nc.vector.scalar_tensor_tensor(out_tile, e[:, :n], 1.0, r[:, :n],
                                   ALU.subtract, ALU.min)


def build_nc(p: P, CH_in, CH_out):
    nc = bacc.Bacc("TRN2", target_bir_lowering=False, debug=False,
                   enable_asserts=True)
    FI, FO, FH = p.F_IN, p.F_OUT, p.F_HID

    dirs = []
    for name, CH in (("in", CH_in), ("out", CH_out)):
        NCH = p.TPC * CH
        dirs.append(dict(
            name=name, CH=CH, NCH=NCH,
            pay=nc.dram_tensor(f"pay_{name}", [128, NCH, FI], BF16,
                               kind="ExternalInput"),
            dstl=nc.dram_tensor(f"dstl_{name}", [128, NCH], F32,
                                kind="ExternalInput"),
            wb=nc.dram_tensor(f"w_{name}", [FI, FO], BF16,
                              kind="ExternalInput"),
            yT=nc.dram_tensor(f"yT_{name}", [128, p.NPAD], F32,
                              kind="ExternalOutput"),
        ))
    iota_d = nc.dram_tensor("iota", [128, 128], BF16, kind="ExternalInput")
    xs_d = nc.dram_tensor("xs", [p.XS_ROWS, FO], BF16, kind="ExternalInput")
    wb1_d = nc.dram_tensor("wb1", [FI + 1, FH], BF16, kind="ExternalInput")
    w2p_d = nc.dram_tensor("w2p", [128, FH // 128 * FO], BF16,
                           kind="ExternalInput")
    zself_d = nc.dram_tensor("zT_self", [128, p.XS_ROWS], F32,
                             kind="ExternalOutput")

    with tile.TileContext(nc) as tc:
        with tc.tile_pool(name="const", bufs=1) as cpool, \
             tc.tile_pool(name="pay", bufs=8) as ppool, \
             tc.tile_pool(name="oh", bufs=16) as opool, \
             tc.tile_pool(name="mid", bufs=3) as mpool, \
             tc.tile_pool(name="selfp", bufs=3) as spool, \
             tc.tile_pool(name="pssum", bufs=3, space="PSUM") as sumpool, \
             tc.tile_pool(name="psy", bufs=1, space="PSUM") as ypool, \
             tc.tile_pool(name="ps1", bufs=2, space="PSUM") as ps1pool, \
             tc.tile_pool(name="ps2", bufs=2, space="PSUM") as ps2pool:

            iota_t = cpool.tile([128, 128], BF16)
            nc.sync.dma_start(iota_t[:], iota_d[:])

            for d in dirs:
                d["dstl_t"] = cpool.tile([128, d["NCH"]], F32,
                                         name=f"dstl_{d['name']}")
                nc.sync.dma_start(d["dstl_t"][:], d["dstl"][:])
                d["wb_t"] = cpool.tile([FI, FO], BF16,
                                       name=f"wb_{d['name']}")
                nc.sync.dma_start(d["wb_t"][:], d["wb"][:])

            # ---- aggregation directions ----
            for d in dirs:
                CH, pay_d, dstl_t = d["CH"], d["pay"], d["dstl_t"]
                for g in range(p.NGRP):
                    t0 = g * p.GT
                    ntg = min(p.GT, p.TPC - t0)
                    n = ntg * 128
                    c0 = t0 * CH
                    payt = ppool.tile([128, p.GT * CH, FI], BF16, tag="payt")
                    nc.sync.dma_start(payt[:, :ntg * CH, :],
                                      pay_d[:, c0:c0 + ntg * CH, :])
                    ps = sumpool.tile([128, 512], F32, tag="ps")
                    for t in range(ntg):
                        for k in range(CH):
                            cc = t * CH + k
                            oh = opool.tile([128, 128], BF16, tag="oh")
                            eng = nc.vector if (cc & 1) else nc.gpsimd
                            eng.tensor_scalar(
                                oh[:], iota_t[:],
                                dstl_t[:, c0 + cc:c0 + cc + 1], None,
                                ALU.is_equal)
                            nc.tensor.matmul(
                                ps[0:FI, 128 * t:128 * (t + 1)],
                                payt[:, cc, :], oh[:],
                                start=(k == 0), stop=(k == CH - 1))
                    s16 = mpool.tile([FI, 512], BF16, tag="s16")
                    nc.scalar.copy(s16[:, :n], ps[0:FI, :n])
                    py = ypool.tile([128, 512], F32, tag="py")
                    nc.tensor.matmul(py[:, :n], d["wb_t"][:], s16[:, :n],
                                     start=True, stop=True)
                    y = mpool.tile([128, 512], F32, tag="y")
                    nc.vector.tensor_scalar_add(y[:, :n], py[:, :n], 0.0)
                    nc.sync.dma_start(d["yT"][:, 128 * t0:128 * t0 + n],
                                      y[:, :n])

            # ---- self branch ----
            wb1 = cpool.tile([FI + 1, FH], BF16)
            nc.sync.dma_start(wb1[:], wb1_d[:])
            w2p = cpool.tile([128, FH // 128 * FO], BF16)
            nc.sync.dma_start(w2p[:], w2p_d[:])
            nk = FH // 128
            for t in range(p.SELF_CH):
                xT = spool.tile([128, 512], BF16, tag="xT")
                nc.scalar.dma_start_transpose(
                    xT[:], xs_d[512 * t:512 * (t + 1), :])
                ps2 = ps2pool.tile([128, 512], F32, tag="ps2")
                for k in range(nk):
                    ps1 = ps1pool.tile([128, 512], F32, tag="ps1")
                    nc.tensor.matmul(ps1[:], wb1[:, 128 * k:128 * (k + 1)],
                                     xT[0:FI + 1, :], start=True, stop=True)
                    hk = spool.tile([128, 512], BF16, tag="hk")
                    _emit_elu(nc, spool, ps1[:], hk[:], 512, BF16)
                    nc.tensor.matmul(ps2[:], w2p[:, FO * k:FO * (k + 1)],
                                     hk[:], start=(k == 0), stop=(k == nk - 1))
                z = spool.tile([128, 512], F32, tag="z")
                nc.vector.tensor_scalar_add(z[:], ps2[:], 0.0)
                nc.sync.dma_start(zself_d[:, 512 * t:512 * (t + 1)], z[:])

    nc.compile()
    return nc


def run(inputs, p: P, trace=False):
    x = np.asarray(inputs["x"], np.float32)
    ei = np.asarray(inputs["edge_index"], np.int64)
    src, dst = ei[0], ei[1]
    x16 = x.astype(BF)

    pay_in, dstl_in, CH_in, recip_in = prep_dir(dst, src, x16, p)
    pay_out, dstl_out, CH_out, recip_out = prep_dir(src, dst, x16, p)

    iota = np.tile(np.arange(128, dtype=np.float32)[None, :],
                   (128, 1)).astype(BF)

    def bf(a):
        return np.asarray(a, np.float32).astype(BF)

    wb1 = np.vstack([inputs["W1"], np.asarray(inputs["b1"])[None, :]])
    W2 = np.asarray(inputs["W2"], np.float32)
    w2p = np.zeros((128, (p.F_HID // 128) * p.F_OUT), np.float32)
    for k in range(p.F_HID // 128):
        w2p[:, k * p.F_OUT:(k + 1) * p.F_OUT] = W2[k * 128:(k + 1) * 128, :]

    # per-core padded bf16 x slice with ones marker col for the self branch
    xb = np.zeros((p.NCORES * p.NPC, 128), np.float32)
    xb[:, :p.F_IN] = x
    xb[:, p.F_IN] = 1.0
    xb16 = xb.astype(BF)

    nc = build_nc(p, CH_in, CH_out)

    in_maps = []
    for c in range(p.NCORES):
        xs = np.zeros((p.XS_ROWS, 128), BF)
        r0 = c * p.NPC
        xs[:p.NPC] = xb16[r0:r0 + p.NPC]
        in_maps.append({
            "pay_in": pay_in[c], "dstl_in": dstl_in[c],
            "pay_out": pay_out[c], "dstl_out": dstl_out[c],
            "w_in": bf(inputs["W_in"]), "w_out": bf(inputs["W_out"]),
            "iota": iota, "xs": xs, "wb1": bf(wb1), "w2p": bf(w2p),
        })

    kw = {}
    if trace:
        kw = dict(trace=True, trace_cores=[0])
    res = run_bass_kernel_spmd(nc, in_maps, core_ids=list(range(p.NCORES)),
                               **kw)

    def gather_out(name):
        return np.concatenate(
            [res.results[c][name][:, :p.NPC].T for c in range(p.NCORES)], 0)

    def elu(z):
        return np.where(z > 0, z, np.expm1(np.minimum(z, 0.0)))

    b_in = np.asarray(inputs["b_in"], np.float32)[None, :]
    b_out = np.asarray(inputs["b_out"], np.float32)[None, :]
    b2 = np.asarray(inputs["b2"], np.float32)[None, :]

    x_in = elu(gather_out("yT_in") * recip_in[:, None] + b_in)
    x_out = elu(gather_out("yT_out") * recip_out[:, None] + b_out)
    x_self = elu(gather_out("zT_self") + b2)
    return (x_in, x_out, x_self), res


def kernel(**inputs):
    p = P()
    (x_in, x_out, x_self), _ = run(inputs, p, trace=False)
    return x_in, x_out, x_self


# revision 8
# speedup vs baseline: 4.2689x; 4.2689x over previous
"""DirSageConv Trainium2 kernel (8 NeuronCores, SPMD).

Strategy: nodes sharded across 8 cores (dst for the "in" direction, src for
the "out" direction); edges partitioned by the aggregation key so the
segment-sum is core-local. The host prepares, per core and direction, an
edge-payload stream (features of the gathered endpoint, bf16, sorted by
aggregation tile and padded per-tile to 128-slot chunks). The device streams
the payload and computes segment-sums as one-hot matmuls accumulated in PSUM
(lhsT = payload chunk [128e x 64f], rhs = one-hot [128e x 128dst] built on
the vector engine via is_equal against an iota row), then multiplies by the
weight matrix. Degree division, bias add, and the outer ELU commute with the
linear ops and are applied on the host. The self branch (two matmuls with an
inner ELU) runs on-device per 512-node chunk; its bias/outer-ELU also finish
on host.
"""
import sys

sys.path.insert(0, "/opt/trn_rl_repo")

import ml_dtypes
import numpy as np

import concourse.bacc as bacc
import concourse.mybir as mybir
from concourse import tile
from concourse.bass_utils import run_bass_kernel_spmd

F32 = mybir.dt.float32
BF16 = mybir.dt.bfloat16

AF = mybir.ActivationFunctionType
ALU = mybir.AluOpType

BF = ml_dtypes.bfloat16


class P:
    def __init__(self, N=100000, F_IN=64, F_OUT=128, F_HID=512, NCORES=8):
        self.N, self.F_IN, self.F_OUT, self.F_HID = N, F_IN, F_OUT, F_HID
        self.NCORES = NCORES
        self.NPC = N // NCORES                    # nodes per core
        self.TPC = -(-self.NPC // 128)            # dst tiles per core (98)
        self.NPAD = self.TPC * 128                # padded nodes per core
        self.GT = 4                               # tiles per psum group
        self.NGRP = -(-self.TPC // self.GT)
        self.SELF_CH = -(-self.NPAD // 512)       # self-branch 512-row chunks
        self.XS_ROWS = 512 * self.SELF_CH


def prep_dir(key, val, x16, p: P):
    """Host prep for one direction: payload stream + one-hot dst columns.

    key: aggregation index per edge (node that receives the sum)
    val: gathered node per edge (features fed into the sum)
    Returns (pay_w [C,128,NCH,64] bf16, dstl_w [C,128,NCH] bf16, CH chunks
    per tile, recip [N] f32).
    """
    E = key.shape[0]
    core = np.minimum(key // p.NPC, p.NCORES - 1)
    kl = key - core * p.NPC
    t = kl >> 7
    dl = (kl & 127).astype(np.float32)

    seg = core * p.TPC + t
    cnt = np.bincount(seg, minlength=p.NCORES * p.TPC)
    CH = int(-(-cnt.max() // 128))                # chunks per tile, uniform
    CAP = CH * 128
    NCH = p.TPC * CH

    order = np.argsort(seg, kind="stable")
    seg_s = seg[order]
    m = np.empty(E, np.bool_)
    m[0] = True
    np.not_equal(seg_s[1:], seg_s[:-1], out=m[1:])
    starts = np.flatnonzero(m)
    sid = np.cumsum(m) - 1
    rank_s = np.arange(E) - starts[sid]
    rank = np.empty(E, np.int64)
    rank[order] = rank_s

    slot = core * (p.TPC * CAP) + t * CAP + rank  # global slot
    pay = np.zeros((p.NCORES * p.TPC * CAP, p.F_IN), BF)
    pay[slot] = x16[val]
    dstl = np.full(p.NCORES * p.TPC * CAP, 255.0, np.float32)
    dstl[slot] = dl

    pay_w = pay.reshape(p.NCORES, NCH, 128, p.F_IN).transpose(0, 2, 1, 3)
    dstl_w = dstl.reshape(p.NCORES, NCH, 128).transpose(0, 2, 1)

    deg = np.bincount(key, minlength=p.N).astype(np.float32)
    recip = 1.0 / np.maximum(deg, 1.0)
    return np.ascontiguousarray(pay_w), np.ascontiguousarray(dstl_w), CH, recip


def _emit_elu(nc, pool, psum_ap, out_tile, n, out_dtype):
    """out = elu(psum) = min(exp(z)-1, relu(z)); psum [128, n]."""
    e = pool.tile([128, n], out_dtype, tag="elu_e")
    nc.scalar.activation(e[:, :n], psum_ap, AF.Exp)
    r = pool.tile([128, n], out_dtype, tag="elu_r")
    nc.vector.tensor_scalar_max(r[:, :n], psum_ap, 0.0)
    nc.vector.scalar_tensor_tensor(out_tile, e[:, :n], 1.0, r[:, :n],
                                   ALU.subtract, ALU.min)


def build_nc(p: P, CH_in, CH_out):
    nc = bacc.Bacc("TRN2", target_bir_lowering=False, debug=False,
                   enable_asserts=True)
    FI, FO, FH = p.F_IN, p.F_OUT, p.F_HID

    dirs = []
    for name, CH in (("in", CH_in), ("out", CH_out)):
        NCH = p.TPC * CH
        dirs.append(dict(
            name=name, CH=CH, NCH=NCH,
            pay=nc.dram_tensor(f"pay_{name}", [128, NCH, FI], BF16,
                               kind="ExternalInput"),
            dstl=nc.dram_tensor(f"dstl_{name}", [128, NCH], F32,
                                kind="ExternalInput"),
            wb=nc.dram_tensor(f"w_{name}", [FI, FO], BF16,
                              kind="ExternalInput"),
            yT=nc.dram_tensor(f"yT_{name}", [128, p.NPAD], F32,
                              kind="ExternalOutput"),
        ))
    iota_d = nc.dram_tensor("iota", [128, 128], BF16, kind="ExternalInput")
    xs_d = nc.dram_tensor("xs", [p.XS_ROWS, FO], BF16, kind="ExternalInput")
    wb1_d = nc.dram_tensor("wb1", [FI + 1, FH], BF16, kind="ExternalInput")
    w2p_d = nc.dram_tensor("w2p", [128, FH // 128 * FO], BF16,
                           kind="ExternalInput")
    zself_d = nc.dram_tensor("zT_self", [128, p.XS_ROWS], F32,
                             kind="ExternalOutput")

    with tile.TileContext(nc) as tc:
        with tc.tile_pool(name="const", bufs=1) as cpool, \
             tc.tile_pool(name="pay", bufs=8) as ppool, \
             tc.tile_pool(name="oh", bufs=16) as opool, \
             tc.tile_pool(name="mid", bufs=3) as mpool, \
             tc.tile_pool(name="selfp", bufs=3) as spool, \
             tc.tile_pool(name="pssum", bufs=3, space="PSUM") as sumpool, \
             tc.tile_pool(name="psy", bufs=1, space="PSUM") as ypool, \
             tc.tile_pool(name="ps1", bufs=2, space="PSUM") as ps1pool, \
             tc.tile_pool(name="ps2", bufs=2, space="PSUM") as ps2pool:

            iota_t = cpool.tile([128, 128], BF16)
            nc.sync.dma_start(iota_t[:], iota_d[:])

            for d in dirs:
                d["dstl_t"] = cpool.tile([128, d["NCH"]], F32,
                                         name=f"dstl_{d['name']}")
                nc.sync.dma_start(d["dstl_t"][:], d["dstl"][:])
                d["wb_t"] = cpool.tile([FI, FO], BF16,
                                       name=f"wb_{d['name']}")
                nc.sync.dma_start(d["wb_t"][:], d["wb"][:])

            # ---- aggregation directions ----
            for d in dirs:
                CH, pay_d, dstl_t = d["CH"], d["pay"], d["dstl_t"]
                for g in range(p.NGRP):
                    t0 = g * p.GT
                    ntg = min(p.GT, p.TPC - t0)
                    n = ntg * 128
                    c0 = t0 * CH
                    payt = ppool.tile([128, p.GT * CH, FI], BF16, tag="payt")
                    nc.sync.dma_start(payt[:, :ntg * CH, :],
                                      pay_d[:, c0:c0 + ntg * CH, :])
                    ps = sumpool.tile([128, 512], F32, tag="ps")
                    for t in range(ntg):
                        for k in range(CH):
                            cc = t * CH + k
                            oh = opool.tile([128, 128], BF16, tag="oh")
                            nc.vector.tensor_scalar(
                                oh[:], iota_t[:],
                                dstl_t[:, c0 + cc:c0 + cc + 1], None,
                                ALU.is_equal)
                            nc.tensor.matmul(
                                ps[0:FI, 128 * t:128 * (t + 1)],
                                payt[:, cc, :], oh[:],
                                start=(k == 0), stop=(k == CH - 1))
                    s16 = mpool.tile([FI, 512], BF16, tag="s16")
                    nc.scalar.copy(s16[:, :n], ps[0:FI, :n])
                    py = ypool.tile([128, 512], F32, tag="py")
                    nc.tensor.matmul(py[:, :n], d["wb_t"][:], s16[:, :n],
                                     start=True, stop=True)
                    y = mpool.tile([128, 512], F32, tag="y")
                    nc.vector.tensor_scalar_add(y[:, :n], py[:, :n], 0.0)
                    nc.sync.dma_start(d["yT"][:, 128 * t0:128 * t0 + n],
                                      y[:, :n])

            # ---- self branch ----
            wb1 = cpool.tile([FI + 1, FH], BF16)
            nc.sync.dma_start(wb1[:], wb1_d[:])
            w2p = cpool.tile([128, FH // 128 * FO], BF16)
            nc.sync.dma_start(w2p[:], w2p_d[:])
            nk = FH // 128
            for t in range(p.SELF_CH):
                xT = spool.tile([128, 512], BF16, tag="xT")
                nc.scalar.dma_start_transpose(
                    xT[:], xs_d[512 * t:512 * (t + 1), :])
                ps2 = ps2pool.tile([128, 512], F32, tag="ps2")
                for k in range(nk):
                    ps1 = ps1pool.tile([128, 512], F32, tag="ps1")
                    nc.tensor.matmul(ps1[:], wb1[:, 128 * k:128 * (k + 1)],
                                     xT[0:FI + 1, :], start=True, stop=True)
                    hk = spool.tile([128, 512], BF16, tag="hk")
                    _emit_elu(nc, spool, ps1[:], hk[:], 512, BF16)
                    nc.tensor.matmul(ps2[:], w2p[:, FO * k:FO * (k + 1)],
                                     hk[:], start=(k == 0), stop=(k == nk - 1))
                z = spool.tile([128, 512], F32, tag="z")
                nc.vector.tensor_scalar_add(z[:], ps2[:], 0.0)
                nc.sync.dma_start(zself_d[:, 512 * t:512 * (t + 1)], z[:])

    nc.compile()
    return nc


def run(inputs, p: P, trace=False):
    x = np.asarray(inputs["x"], np.float32)
    ei = np.asarray(inputs["edge_index"], np.int64)
    src, dst = ei[0], ei[1]
    x16 = x.astype(BF)

    pay_in, dstl_in, CH_in, recip_in = prep_dir(dst, src, x16, p)
    pay_out, dstl_out, CH_out, recip_out = prep_dir(src, dst, x16, p)

    iota = np.tile(np.arange(128, dtype=np.float32)[None, :],
                   (128, 1)).astype(BF)

    def bf(a):
        return np.asarray(a, np.float32).astype(BF)

    wb1 = np.vstack([inputs["W1"], np.asarray(inputs["b1"])[None, :]])
    W2 = np.asarray(inputs["W2"], np.float32)
    w2p = np.zeros((128, (p.F_HID // 128) * p.F_OUT), np.float32)
    for k in range(p.F_HID // 128):
        w2p[:, k * p.F_OUT:(k + 1) * p.F_OUT] = W2[k * 128:(k + 1) * 128, :]

    # per-core padded bf16 x slice with ones marker col for the self branch
    xb = np.zeros((p.NCORES * p.NPC, 128), np.float32)
    xb[:, :p.F_IN] = x
    xb[:, p.F_IN] = 1.0
    xb16 = xb.astype(BF)

    nc = build_nc(p, CH_in, CH_out)

    in_maps = []
    for c in range(p.NCORES):
        xs = np.zeros((p.XS_ROWS, 128), BF)
        r0 = c * p.NPC
        xs[:p.NPC] = xb16[r0:r0 + p.NPC]
        in_maps.append({
            "pay_in": pay_in[c], "dstl_in": dstl_in[c],
            "pay_out": pay_out[c], "dstl_out": dstl_out[c],
            "w_in": bf(inputs["W_in"]), "w_out": bf(inputs["W_out"]),
            "iota": iota, "xs": xs, "wb1": bf(wb1), "w2p": bf(w2p),
        })

    kw = {}
    if trace:
        kw = dict(trace=True, trace_cores=[0])
    res = run_bass_kernel_spmd(nc, in_maps, core_ids=list(range(p.NCORES)),
                               **kw)

    def gather_out(name):
        return np.concatenate(
            [res.results[c][name][:, :p.NPC].T for c in range(p.NCORES)], 0)

    def elu(z):
        return np.where(z > 0, z, np.expm1(np.minimum(z, 0.0)))

    b_in = np.asarray(inputs["b_in"], np.float32)[None, :]
    b_out = np.asarray(inputs["b_out"], np.float32)[None, :]
    b2 = np.asarray(inputs["b2"], np.float32)[None, :]

    x_in = elu(gather_out("yT_in") * recip_in[:, None] + b_in)
    x_out = elu(gather_out("yT_out") * recip_out[:, None] + b_out)
    x_self = elu(gather_out("zT_self") + b2)
    return (x_in, x_out, x_self), res


def kernel(**inputs):
    p = P()
    (x_in, x_out, x_self), _ = run(inputs, p, trace=False)
    return x_in, x_out, x_self
